# revision 85
# baseline (speedup 1.0000x reference)
"""Self-contained Trainium2 Bass kernel for a 6-layer dense transformer.

Model (from reference): DIM=1024, DEPTH=6, HEADS=16, FF=4096, x [2,1024,1024],
relative_position_bias [1,16,1024,1024], pre-norm attention+FFN, exact GELU.

Strategy: sequence-parallel over 8 NeuronCores. Rows = flatten(batch, seq) =
2048; each core owns 256 rows (batch b = core//4, seq chunk core%4). Weights
are replicated and streamed from HBM as bf16 via the scalar-engine HWDGE ring
(keeps the SP ring free for kv bounces and the Pool queue free for
collectives). Activations are CHANNEL-major (transposed: [D, rows]).

Per-layer K/V AllGather (fp8, across the 4 cores of each batch) is split into
two row-half collectives and software-pipelined: the layer-l w2 matmul runs as
two half-row passes, with the next layer's LN1-A / K-A / V-A / AllGather-A
emitted between them, so AG-A overlaps w2-B + LN1-B + K-B/V-B + q and AG-B
overlaps q + the even-key-tile half of attention (phase-major attention with
SBUF staging of the phase-A partial sums).

Softmax runs without max-subtraction (scores provably small); bias folded in
as attn = exp(s)*exp(bias) with exp(bias) precomputed in SBUF (fp8, slot-major
so each attention phase reads a contiguous slice). K/V travel as fp8; scores
and attention weights stay bf16/f32. Denominator comes from a ones-column
appended to V. Output streams out per channel-chunk as the last layer's w2
completes; host assembles the full array.
"""
import sys
sys.path.insert(0, "/opt/trn_rl_repo")

import numpy as np

import concourse.bass as bass
import concourse.tile as tile
from concourse import bacc, mybir

P = 128
D = 1024
DT = 8            # D / P tiles
DEPTH = 6
HEADS = 16
DH = 64
FF = 4096
FFT = 32          # FF / P tiles
R = 256           # rows per core
HR = 128          # rows per half
B = 2
SEQ = 1024
N_CORES = 8
EPS = 1e-5
SCALE = DH ** -0.5
RG = [[0, 1, 2, 3], [4, 5, 6, 7]]

F32 = mybir.dt.float32
BF16 = mybir.dt.bfloat16
FP8 = mybir.dt.float8e4
AX = mybir.AluOpType
AF = mybir.ActivationFunctionType

KHALF = DT * P * HR   # k elems per half-row bounce (131072)
VHALF = P * D         # v elems per half-row bounce (131072)
NQKV_CH = 12
NOUT_CH = 4
NW1_CH = 16


def _bcast_mid(ap, n):
    """View a [P, N] AP as [P, n, N] with a 0-stride middle dim."""
    return bass.AP(tensor=ap.tensor, offset=ap.offset,
                   ap=[list(ap.ap[0]), [0, n], list(ap.ap[1])])


def build_nc(repeat=1):
    nc = bacc.Bacc("TRN2", target_bir_lowering=False, debug=False,
                   num_devices=N_CORES)

    xT_ext = nc.dram_tensor("xT", [D, R], F32, kind="ExternalInput")
    biasT_ext = nc.dram_tensor("biasT", [HEADS, 2, P, DT // 2, R], BF16,
                               kind="ExternalInput")
    w_q_ext = nc.dram_tensor("w_q", [DEPTH, 4, P, DT, 2 * P], FP8,
                             kind="ExternalInput")
    w_kv_ext = nc.dram_tensor("w_kv", [DEPTH, 8, P, DT, 2 * P], FP8,
                              kind="ExternalInput")
    w_out_ext = nc.dram_tensor("w_out", [DEPTH, NOUT_CH, P, DT, 2 * P], FP8,
                               kind="ExternalInput")
    w1_ext = nc.dram_tensor("w1", [DEPTH, NW1_CH, P, DT, 2 * P], BF16,
                            kind="ExternalInput")
    w2_ext = nc.dram_tensor("w2", [DEPTH, 4, 4, P, DT, 2 * P], BF16,
                            kind="ExternalInput")
    b_out_ext = nc.dram_tensor("b_out", [DEPTH, D], F32, kind="ExternalInput")
    ln1_g_ext = nc.dram_tensor("ln1_g", [DEPTH, D], F32, kind="ExternalInput")
    ln1_b_ext = nc.dram_tensor("ln1_b", [DEPTH, D], F32, kind="ExternalInput")
    ln2_g_ext = nc.dram_tensor("ln2_g", [DEPTH, D], F32, kind="ExternalInput")
    ln2_b_ext = nc.dram_tensor("ln2_b", [DEPTH, D], F32, kind="ExternalInput")
    b1_ext = nc.dram_tensor("b1", [DEPTH, FF], F32, kind="ExternalInput")
    b2_ext = nc.dram_tensor("b2", [DEPTH, D], F32, kind="ExternalInput")
    outT_ext = nc.dram_tensor("outT", [D, R], F32, kind="ExternalOutput")

    from contextlib import ExitStack
    with tile.TileContext(nc) as tc, ExitStack() as ctx:
        ep = ctx.enter_context
        singles = ep(tc.tile_pool(name="singles", bufs=1))
        params = ep(tc.tile_pool(name="params", bufs=2))
        statp = ep(tc.tile_pool(name="stat", bufs=2))
        hTp = ep(tc.tile_pool(name="hTp", bufs=2))
        qTp = ep(tc.tile_pool(name="qTp", bufs=1))
        kvst = ep(tc.tile_pool(name="kvst", bufs=2))
        ktp = ep(tc.tile_pool(name="ktp", bufs=1))
        vpp = ep(tc.tile_pool(name="vpp", bufs=1))
        attnp = ep(tc.tile_pool(name="attnp", bufs=3))
        oap = ep(tc.tile_pool(name="oap", bufs=1))
        oTp = ep(tc.tile_pool(name="oTp", bufs=1))
        gTp = ep(tc.tile_pool(name="gTp", bufs=1))
        kvwcp = ep(tc.tile_pool(name="kvwcp", bufs=8))
        wcp = ep(tc.tile_pool(name="wcp", bufs=15))
        vecp = ep(tc.tile_pool(name="vecp", bufs=4))
        psmm = ep(tc.tile_pool(name="psmm", bufs=3, space="PSUM"))
        psav = ep(tc.tile_pool(name="psav", bufs=2, space="PSUM"))
        psbc = ep(tc.tile_pool(name="psbc", bufs=2, space="PSUM"))
        psst = ep(tc.tile_pool(name="psst", bufs=1, space="PSUM"))
        dram = ep(tc.tile_pool(name="dram", bufs=4, space="DRAM"))
        if True:
            # ---- persistent tiles ----
            xT = singles.tile([P, DT, R], F32, tag="xT")
            EB = singles.tile([P, HEADS, DT, R], FP8, tag="EB")
            ones_red = singles.tile([P, 1], BF16, tag="ones_red")
            ones_k1 = singles.tile([1, P], BF16, tag="ones_k1")
            sel2 = singles.tile([33, P], BF16, tag="sel2")
            den = singles.tile([33, DT, R], BF16, tag="den")
            nc.vector.memset(ones_red[:], 1.0)
            nc.vector.memset(ones_k1[:], 1.0)
            nc.vector.memset(sel2[:], 0.0)
            nc.vector.memset(sel2[0:1, 0:DH], 1.0)
            nc.vector.memset(sel2[32:33, DH:P], 1.0)
            nc.vector.memset(den[:], 1.0)
            eps_t = singles.tile([1, 1], F32, tag="eps")
            nc.vector.memset(eps_t[:], EPS)

            for t in range(DT):
                nc.sync.dma_start(
                    out=xT[:, t],
                    in_=xT_ext.ap().rearrange("(t p) r -> p t r", p=P)[:, t])

            # EB = exp(bias^T), bf16, slot-major (even key tiles then odd).
            # Loads ride the scalar-engine HWDGE ring so they overlap the
            # layer-0 chains without blocking the SP ring; emitted by
            # emit_eb_load() after the layer-0 chains so they don't delay them.
            def emit_eb_load():
                for h in range(HEADS):
                    for hf in range(2):
                        tmp = statp.tile([P, DT // 2, R], BF16, tag="btmp",
                                         name=f"btmp_{h}_{hf}")
                        nc.scalar.dma_start(out=tmp[:], in_=biasT_ext.ap()[h, hf])
                        nc.scalar.activation(EB[:, h, hf * 4:(hf + 1) * 4],
                                             tmp[:], AF.Exp)

            def ln_alloc(tag):
                xb = statp.tile([P, DT, R], BF16, tag="stat", name=f"xb_{tag}")
                sq = statp.tile([P, DT, R], BF16, tag="stat", name=f"sq_{tag}")
                ps_st = psst.tile([33, R], F32, tag="st", name=f"st_{tag}")
                return xb, sq, ps_st

            def ln_contrib(st, t, hs):
                """Accumulate LN stats for channel-tile t, row slice hs."""
                xb, sq, ps_st = st
                nc.vector.tensor_copy(xb[:, t, hs], xT[:, t, hs])
                nc.vector.tensor_mul(sq[:, t, hs], xb[:, t, hs], xb[:, t, hs])
                nc.tensor.matmul(ps_st[0:1, hs], ones_red[:], xb[:, t, hs],
                                 start=(t == 0), stop=(t == DT - 1))
                nc.tensor.matmul(ps_st[32:33, hs], ones_red[:], sq[:, t, hs],
                                 start=(t == 0), stop=(t == DT - 1))

            def ln_finish(st, g_sb, b_sb, out_hT, hs, tag):
                """LN over channel (partition) axis for row slice hs."""
                xb, sq, ps_st = st
                w = hs.stop - hs.start
                mu = vecp.tile([1, w], F32, tag="vec", name=f"mu_{tag}")
                var = vecp.tile([1, w], F32, tag="vec", name=f"var_{tag}")
                ms = vecp.tile([1, w], F32, tag="vec", name=f"ms_{tag}")
                rstd = vecp.tile([1, w], F32, tag="vec", name=f"rstd_{tag}")
                nc.vector.tensor_scalar_mul(mu[:], ps_st[0:1, hs], 1.0 / D)
                nc.vector.tensor_scalar_mul(var[:], ps_st[32:33, hs], 1.0 / D)
                nc.vector.tensor_mul(ms[:], mu[:], mu[:])
                nc.vector.tensor_sub(var[:], var[:], ms[:])
                nc.scalar.activation(var[:], var[:], AF.Sqrt, bias=eps_t[:])
                nc.vector.reciprocal(rstd[:], var[:])
                ones_f = vecp.tile([1, P], F32, tag="vec16", bufs=2, name=f"onesf_{tag}")
                nc.vector.memset(ones_f[:], 1.0)
                ps_mu = psbc.tile([P, w], F32, tag="bc", name=f"psmu_{tag}")
                ps_rs = psbc.tile([P, w], F32, tag="bc", name=f"psrs_{tag}")
                nc.tensor.matmul(ps_mu[:], ones_f[:], mu[:], start=True, stop=True)
                nc.tensor.matmul(ps_rs[:], ones_f[:], rstd[:], start=True, stop=True)
                mub = statp.tile([P, w], BF16, tag="statv", name=f"mub_{tag}")
                rsb = statp.tile([P, w], BF16, tag="statv", name=f"rsb_{tag}")
                nc.vector.tensor_copy(mub[:], ps_mu[:])
                nc.vector.tensor_copy(rsb[:], ps_rs[:])
                nc.vector.tensor_sub(xb[:, :, hs], xT[:, :, hs],
                                     _bcast_mid(mub[:], DT))
                nc.vector.tensor_mul(xb[:, :, hs], xb[:, :, hs],
                                     _bcast_mid(rsb[:], DT))
                for t in range(DT):
                    nc.vector.tensor_scalar(
                        out_hT[:, t, hs], xb[:, t, hs],
                        g_sb[:, t:t + 1], b_sb[:, t:t + 1],
                        op0=AX.mult, op1=AX.add)

            def load_params(l):
                g1 = params.tile([P, DT], F32, tag="g1", name=f"g1_{l}")
                b1p = params.tile([P, DT], F32, tag="b1p", name=f"b1p_{l}")
                g2 = params.tile([P, DT], F32, tag="g2", name=f"g2_{l}")
                b2p = params.tile([P, DT], F32, tag="b2p", name=f"b2p_{l}")
                bo = params.tile([P, DT], F32, tag="bo", name=f"bo_{l}")
                bf = params.tile([P, FFT], F32, tag="bf", name=f"bf_{l}")
                b2f = params.tile([P, DT], F32, tag="b2f", name=f"b2f_{l}")
                for t, ext in ((g1, ln1_g_ext), (b1p, ln1_b_ext),
                               (g2, ln2_g_ext), (b2p, ln2_b_ext),
                               (bo, b_out_ext), (bf, b1_ext), (b2f, b2_ext)):
                    nc.sync.dma_start(out=t[:],
                                      in_=ext.ap()[l].rearrange("(t p) -> p t", p=P))
                return g1, b1p, g2, b2p, bo, bf, b2f

            def load_kvwc(l):
                tiles = []
                for ch in range(8):
                    wc = kvwcp.tile([P, DT, 2 * P], FP8, tag="kvwc",
                                    name=f"wckv_{l}_{ch}")
                    nc.scalar.dma_start(out=wc[:], in_=w_kv_ext.ap()[l, ch])
                    tiles.append(wc)
                return tiles

            def alloc_layer_tiles(l):
                hT = hTp.tile([P, DT, R], BF16, tag="hT", name=f"hT_{l}")
                kst = kvst.tile([P, 2, DT, HR], FP8, tag="kv", name=f"kst_{l}")
                vst = kvst.tile([P, 2, D], FP8, tag="kv", name=f"vst_{l}")
                kvi = [dram.tile([KHALF + VHALF], FP8, tag="kv_in",
                                 name=f"kvi_{l}_{h}") for h in range(2)]
                kvo = [dram.tile([4, KHALF + VHALF], FP8, tag="kv_out",
                                 name=f"kvo_{l}_{h}") for h in range(2)]
                return hT, kst, vst, kvi, kvo

            def chain_half(l, h, lt, ln1_st, g1, b1p, kvwc):
                """LN1 + K + V + bounce + AllGather for row-half h of layer l."""
                hT, kst, vst, kvi, kvo = lt
                hs = slice(h * HR, (h + 1) * HR)
                ln_finish(ln1_st, g1, b1p, hT, hs, f"l{l}h{h}")
                # K columns (w_qkv chunks 4..7), half rows
                for ci, ch in enumerate(range(4, 8)):
                    wc = kvwc[ci]
                    for sub in range(2):
                        c = ci * 2 + sub
                        ps = psmm.tile([P, HR], F32, tag="mm",
                                       name=f"psk_{l}_{h}_{ch}_{sub}")
                        for kt in range(DT):
                            nc.tensor.matmul(ps[:], wc[:, kt, sub * P:(sub + 1) * P],
                                             hT[:, kt, hs],
                                             start=(kt == 0), stop=(kt == DT - 1))
                        nc.vector.tensor_copy(kst[:, h, c], ps[:])
                nc.sync.dma_start(
                    out=bass.AP(tensor=kvi[h][:].tensor, offset=kvi[h][:].offset,
                                ap=[[HR, P], [P * HR, DT], [1, HR]]),
                    in_=kst[:, h])
                # V rows for this half (rt == h), row-major
                for ch in range(8, 12):
                    wc = kvwc[ch - 8 + 4]
                    ps = psmm.tile([P, 2 * P], F32, tag="mm",
                                   name=f"psv_{l}_{h}_{ch}")
                    for kt in range(DT):
                        nc.tensor.matmul(ps[:], hT[:, kt, hs], wc[:, kt],
                                         start=(kt == 0), stop=(kt == DT - 1))
                    nc.vector.tensor_copy(
                        vst[:, h, (ch - 8) * 256:(ch - 7) * 256], ps[:])
                nc.sync.dma_start(
                    out=bass.AP(tensor=kvi[h][:].tensor,
                                offset=kvi[h][:].offset + KHALF,
                                ap=[[D, P], [1, D]]),
                    in_=vst[:, h])
                nc.gpsimd.collective_compute(
                    "AllGather", AX.bypass, replica_groups=RG,
                    ins=[kvi[h][:]], outs=[kvo[h][:]])

            def emit_gather_loads(l, h, lt, KT2, Vp):
                _, _, _, _, kvo = lt
                for r in range(4):
                    nc.gpsimd.dma_start(
                        out=KT2[:, :, r * R + h * HR: r * R + (h + 1) * HR],
                        in_=kvo[h][r, :KHALF].rearrange("(c p k) -> p c k",
                                                        p=P, k=HR))
                    nc.gpsimd.dma_start(
                        out=Vp[:, r * 2 + h, :, 0:DH],
                        in_=kvo[h][r, KHALF:].rearrange("(p a j) -> p a j",
                                                        p=P, j=DH))

            # ================= pipelined prologue =================
            g1, b1p, g2, b2p, bo, bf, b2f = load_params(0)
            kvwc = load_kvwc(0)
            lt = alloc_layer_tiles(0)
            ln1_st = ln_alloc("l0a")
            for t in range(DT):
                ln_contrib(ln1_st, t, slice(0, HR))
            chain_half(0, 0, lt, ln1_st, g1, b1p, kvwc)
            for t in range(DT):
                ln_contrib(ln1_st, t, slice(HR, R))
            chain_half(0, 1, lt, ln1_st, g1, b1p, kvwc)
            emit_eb_load()

            for _rep in range(repeat):
                for l in range(DEPTH):
                    hT, kst, vst, kvi, kvo = lt

                    # prefetch next layer's k/v weight chunks during attention
                    kvwc_n = load_kvwc(l + 1) if l < DEPTH - 1 else None

                    # ---- q projection (overlaps AG-B of this layer) ----
                    qT = qTp.tile([P, DT, R], BF16, tag="qT", name=f"qT_{l}")
                    for ch in range(4):
                        wc = wcp.tile([P, DT, 2 * P], FP8, tag="wc",
                                      name=f"wcq_{l}_{ch}")
                        nc.scalar.dma_start(out=wc[:], in_=w_q_ext.ap()[l, ch])
                        for sub in range(2):
                            c = ch * 2 + sub
                            ps = psmm.tile([P, R], F32, tag="mm",
                                           name=f"psq_{l}_{ch}_{sub}")
                            for kt in range(DT):
                                nc.tensor.matmul(ps[:], wc[:, kt, sub * P:(sub + 1) * P],
                                                 hT[:, kt], start=(kt == 0),
                                                 stop=(kt == DT - 1))
                            nc.vector.tensor_copy(qT[:, c], ps[:])

                    # ---- gathered K^T / V+ones into SBUF ----
                    KT2 = ktp.tile([P, DT, SEQ], FP8, tag="KT2", name=f"KT2_{l}")
                    Vp = vpp.tile([P, DT, HEADS, DH + 1], FP8, tag="Vp",
                                  name=f"Vp_{l}")
                    nc.vector.memset(Vp[:, :, :, DH:DH + 1], 1.0)
                    emit_gather_loads(l, 0, lt, KT2, Vp)
                    emit_gather_loads(l, 1, lt, KT2, Vp)

                    # ---- attention: phase A (even key tiles), staged ----
                    oA = oap.tile([DH + 1, HEADS, R], BF16, tag="oA",
                                  name=f"oA_{l}")
                    for h in range(HEADS):
                        pb = (h % 2) * DH
                        hp = h // 2
                        at = attnp.tile([P, 4, R], BF16, tag="attn",
                                        name=f"atA_{l}_{h}")
                        ps_o = psav.tile([DH + 1, R], F32, tag="av",
                                         name=f"psoA_{l}_{h}")
                        for k2 in range(2):
                            ps_s = psmm.tile([P, 2 * R], F32, tag="mm",
                                             name=f"pssA_{l}_{h}_{k2}")
                            for j in range(2):
                                kt = (2 * k2 + j) * 2
                                nc.tensor.matmul(
                                    ps_s[:, j * R:(j + 1) * R],
                                    KT2[pb:pb + DH, hp, kt * P:(kt + 1) * P],
                                    qT[pb:pb + DH, hp], start=True, stop=True)
                            nc.scalar.activation(
                                at[:, 2 * k2:2 * k2 + 2].rearrange("p a b -> p (a b)"),
                                ps_s[:], AF.Exp, scale=SCALE)
                        nc.vector.tensor_mul(at[:], at[:], EB[:, h, 0:4])
                        for s in range(4):
                            nc.tensor.matmul(ps_o[:], Vp[:, 2 * s, h], at[:, s],
                                             start=(s == 0), stop=(s == 3))
                        nc.vector.tensor_copy(oA[:, h], ps_o[:])

                    # ---- attention: phase B (odd key tiles) + combine ----
                    # Per-head work only accumulates the unnormalized sum and
                    # drops the denominator into row h%2 / column hp of `den`;
                    # normalization happens once for all heads afterwards.
                    oT = oTp.tile([P, DT, R], BF16, tag="oT", name=f"oT_{l}")
                    for h in range(HEADS):
                        pb = (h % 2) * DH
                        hp = h // 2
                        at = attnp.tile([P, 4, R], BF16, tag="attn",
                                        name=f"atB_{l}_{h}")
                        ps_o = psav.tile([DH + 1, R], F32, tag="av",
                                         name=f"psoB_{l}_{h}")
                        for k2 in range(2):
                            ps_s = psmm.tile([P, 2 * R], F32, tag="mm",
                                             name=f"pssB_{l}_{h}_{k2}")
                            for j in range(2):
                                kt = (2 * k2 + j) * 2 + 1
                                nc.tensor.matmul(
                                    ps_s[:, j * R:(j + 1) * R],
                                    KT2[pb:pb + DH, hp, kt * P:(kt + 1) * P],
                                    qT[pb:pb + DH, hp], start=True, stop=True)
                            nc.scalar.activation(
                                at[:, 2 * k2:2 * k2 + 2].rearrange("p a b -> p (a b)"),
                                ps_s[:], AF.Exp, scale=SCALE)
                        nc.vector.tensor_mul(at[:], at[:], EB[:, h, 4:8])
                        for s in range(4):
                            nc.tensor.matmul(ps_o[:], Vp[:, 2 * s + 1, h], at[:, s],
                                             start=(s == 0), stop=(s == 3))
                        with nc.allow_low_precision(reason="softmax denom bf16"):
                            nc.vector.tensor_add(
                                den[(h % 2) * 32:(h % 2) * 32 + 1, hp],
                                ps_o[DH:DH + 1], oA[DH:DH + 1, h])
                        nc.vector.tensor_add(oT[pb:pb + DH, hp], ps_o[0:DH],
                                             oA[0:DH, h])
                        if h % 2 == 1:
                            # normalize this head pair now so the chain hides
                            # under the remaining heads' matmuls
                            with nc.allow_low_precision(reason="softmax denom bf16"):
                                nc.vector.reciprocal(den[0:1, hp], den[0:1, hp])
                                nc.vector.reciprocal(den[32:33, hp],
                                                     den[32:33, hp])
                            ps_b = psbc.tile([P, R], F32, tag="bc",
                                             name=f"ps_b_{l}_{hp}")
                            nc.tensor.matmul(ps_b[:], sel2[:, :], den[:, hp],
                                             start=True, stop=True)
                            nc.vector.tensor_mul(oT[:, hp], oT[:, hp], ps_b[:])

                    # ---- attn out projection + residual (+LN2 stats) ----
                    ln2_st = ln_alloc(f"l{l}b")
                    for ch in range(NOUT_CH):
                        wc = wcp.tile([P, DT, 2 * P], FP8, tag="wc",
                                      name=f"wco_{l}_{ch}")
                        nc.scalar.dma_start(out=wc[:], in_=w_out_ext.ap()[l, ch])
                        for sub in range(2):
                            c = ch * 2 + sub
                            ps = psmm.tile([P, R], F32, tag="mm",
                                           name=f"pso2_{l}_{ch}_{sub}")
                            for kt in range(DT):
                                nc.tensor.matmul(ps[:], wc[:, kt, sub * P:(sub + 1) * P],
                                                 oT[:, kt], start=(kt == 0),
                                                 stop=(kt == DT - 1))
                            nc.vector.scalar_tensor_tensor(
                                out=xT[:, c], in0=ps[:], scalar=bo[:, c:c + 1],
                                in1=xT[:, c], op0=AX.add, op1=AX.add)
                            ln_contrib(ln2_st, c, slice(0, R))

                    # ---- LN2 + FFN ----
                    h2 = hTp.tile([P, DT, R], BF16, tag="hT", name=f"h2_{l}")
                    gT = gTp.tile([P, FFT, R], BF16, tag="gT", name=f"gT_{l}")

                    def w1_pass(l, hs, tag):
                        for ch in range(NW1_CH):
                            wc = wcp.tile([P, DT, 2 * P], BF16, tag="wc",
                                          name=f"wc1_{l}_{tag}_{ch}")
                            nc.scalar.dma_start(out=wc[:], in_=w1_ext.ap()[l, ch])
                            for sub in range(2):
                                f = ch * 2 + sub
                                ps = psmm.tile([P, hs.stop - hs.start], F32,
                                               tag="mm", name=f"psf_{l}_{tag}_{ch}_{sub}")
                                for kt in range(DT):
                                    nc.tensor.matmul(
                                        ps[:], wc[:, kt, sub * P:(sub + 1) * P],
                                        h2[:, kt, hs], start=(kt == 0),
                                        stop=(kt == DT - 1))
                                nc.scalar.activation(gT[:, f, hs], ps[:], AF.Gelu,
                                                     bias=bf[:, f:f + 1])

                    if l < DEPTH - 1:
                        # LN2/w1/w2 as two half-row passes; next layer's
                        # chain-A emitted between them so its AllGather
                        # overlaps the entire B pass + q + attention-A.
                        g1n, b1pn, g2n, b2pn, bon, bfn, b2fn = load_params(l + 1)
                        lt_n = alloc_layer_tiles(l + 1)
                        ln_finish(ln2_st, g2, b2p, h2, slice(0, HR), f"l{l}b0")
                        ln_finish(ln2_st, g2, b2p, h2, slice(HR, R), f"l{l}b1")
                        ln1_n = ln_alloc(f"l{l + 1}a")
                        for hf in range(2):
                            hs = slice(hf * HR, (hf + 1) * HR)
                            w1_pass(l, hs, f"h{hf}")
                            for cp in range(4):
                                pss = [psmm.tile([P, HR], F32, tag="mm",
                                                 name=f"ps_mm2_{l}_{hf}_{cp}_{i}")
                                       for i in range(2)]
                                for ktg in range(4):
                                    wc = wcp.tile([P, DT, 2 * P], BF16, tag="wc",
                                                  name=f"wc2_{l}_{hf}_{cp}_{ktg}")
                                    nc.scalar.dma_start(out=wc[:],
                                                        in_=w2_ext.ap()[l, cp, ktg])
                                    for sub in range(2):
                                        for k8 in range(DT):
                                            nc.tensor.matmul(
                                                pss[sub][:],
                                                wc[:, k8, sub * P:(sub + 1) * P],
                                                gT[:, ktg * 8 + k8, hs],
                                                start=(ktg == 0 and k8 == 0),
                                                stop=(ktg == 3 and k8 == DT - 1))
                                for sub in range(2):
                                    c = cp * 2 + sub
                                    nc.vector.scalar_tensor_tensor(
                                        out=xT[:, c, hs], in0=pss[sub][:],
                                        scalar=b2f[:, c:c + 1],
                                        in1=xT[:, c, hs], op0=AX.add, op1=AX.add)
                                    ln_contrib(ln1_n, c, hs)
                            chain_half(l + 1, hf, lt_n, ln1_n, g1n, b1pn, kvwc_n)
                        lt = lt_n
                        g1, b1p, g2, b2p, bo, bf, b2f = (
                            g1n, b1pn, g2n, b2pn, bon, bfn, b2fn)
                    else:
                        ln_finish(ln2_st, g2, b2p, h2, slice(0, R), f"l{l}b")
                        w1_pass(l, slice(0, R), "full")
                        for cp in range(4):
                            pss = [psmm.tile([P, R], F32, tag="mm",
                                             name=f"ps_mm2_{l}_{cp}_{i}")
                                   for i in range(2)]
                            for ktg in range(4):
                                wc = wcp.tile([P, DT, 2 * P], BF16, tag="wc",
                                              name=f"wc2_{l}_{cp}_{ktg}")
                                nc.scalar.dma_start(out=wc[:],
                                                    in_=w2_ext.ap()[l, cp, ktg])
                                for sub in range(2):
                                    for k8 in range(DT):
                                        nc.tensor.matmul(
                                            pss[sub][:],
                                            wc[:, k8, sub * P:(sub + 1) * P],
                                            gT[:, ktg * 8 + k8],
                                            start=(ktg == 0 and k8 == 0),
                                            stop=(ktg == 3 and k8 == DT - 1))
                            for sub in range(2):
                                c = cp * 2 + sub
                                nc.vector.scalar_tensor_tensor(
                                    out=xT[:, c], in0=pss[sub][:],
                                    scalar=b2f[:, c:c + 1],
                                    in1=xT[:, c], op0=AX.add, op1=AX.add)
                                nc.sync.dma_start(
                                    out=outT_ext.ap().rearrange(
                                        "(t p) r -> p t r", p=P)[:, c],
                                    in_=xT[:, c])

    nc.compile()
    return nc


def make_in_maps(inputs):
    from ml_dtypes import bfloat16
    x = np.ascontiguousarray(np.asarray(inputs["x"], dtype=np.float32))
    bias = np.asarray(inputs["relative_position_bias"], dtype=np.float32)

    def pack(w, nch):
        # [DEPTH, 128*DT rows, 256*nch cols] -> [DEPTH, nch, 128, DT, 256]
        w = np.asarray(w, dtype=np.float32)
        return np.ascontiguousarray(
            w.reshape(DEPTH, DT, P, nch, 2 * P).transpose(0, 3, 2, 1, 4)
            .astype(bfloat16))

    w2 = np.asarray(inputs["w2"], dtype=np.float32)
    w2p = np.ascontiguousarray(
        w2.reshape(DEPTH, 4, DT, P, 4, 2 * P).transpose(0, 4, 1, 3, 2, 5)
        .astype(bfloat16))

    from ml_dtypes import float8_e4m3fn
    wqkv_p = pack(inputs["w_qkv"], NQKV_CH)
    shared = {
        "w_q": np.ascontiguousarray(
            wqkv_p[:, 0:4].astype(np.float32).astype(float8_e4m3fn)),
        "w_kv": np.ascontiguousarray(
            wqkv_p[:, 4:12].astype(np.float32).astype(float8_e4m3fn)),
        "w_out": np.ascontiguousarray(
            pack(inputs["w_out"], NOUT_CH).astype(np.float32)
            .astype(float8_e4m3fn)),
        "w1": pack(inputs["w1"], NW1_CH),
        "w2": w2p,
        "b_out": np.ascontiguousarray(inputs["b_out"], dtype=np.float32),
        "ln1_g": np.ascontiguousarray(inputs["ln1_g"], dtype=np.float32),
        "ln1_b": np.ascontiguousarray(inputs["ln1_b"], dtype=np.float32),
        "ln2_g": np.ascontiguousarray(inputs["ln2_g"], dtype=np.float32),
        "ln2_b": np.ascontiguousarray(inputs["ln2_b"], dtype=np.float32),
        "b1": np.ascontiguousarray(inputs["b1"], dtype=np.float32),
        "b2": np.ascontiguousarray(inputs["b2"], dtype=np.float32),
    }
    PERM = [0, 2, 4, 6, 1, 3, 5, 7]  # EB slot s holds absolute key tile PERM[s]
    in_maps = []
    for c in range(N_CORES):
        b, s0 = c // 4, (c % 4) * R
        m = dict(shared)
        m["xT"] = np.ascontiguousarray(x[b, s0:s0 + R, :].T)
        bt = bias[0, :, s0:s0 + R, :].transpose(0, 2, 1)  # [16, 1024 keys, 256]
        tiles = bt.reshape(HEADS, DT, P, R)[:, PERM]      # slot-major key tiles
        m["biasT"] = np.ascontiguousarray(
            tiles.reshape(HEADS, 2, DT // 2, P, R).transpose(0, 1, 3, 2, 4)
            .astype(bfloat16))
        in_maps.append(m)
    return in_maps


_NC_CACHE = {}


def kernel(**inputs):
    from concourse.bass_utils import run_bass_kernel_spmd
    if "nc" not in _NC_CACHE:
        _NC_CACHE["nc"] = build_nc()
    nc = _NC_CACHE["nc"]
    in_maps = make_in_maps(inputs)
    res = run_bass_kernel_spmd(nc, in_maps, core_ids=list(range(N_CORES)))
    out = np.empty((B, SEQ, D), dtype=np.float32)
    for c in range(N_CORES):
        b, s0 = c // 4, (c % 4) * R
        out[b, s0:s0 + R, :] = res.results[c]["outT"].T
    return out
